# revision 60
# baseline (speedup 1.0000x reference)
"""TRN2 Bass kernel for nn_GTLayer (ELL sparse attention, N=50000, K=16).

Sharding: nodes are ranked globally by active-neighbor count and dealt
round-robin to the 8 NeuronCores (6250/core, padded to 6272 = 49x128), so
every core's tile t holds nodes of near-identical counts and the per-tile
compacted neighbor budget K_t (avg ~8.6 vs 16) is shared across cores.

phase 1 (per 128-node tile, software-pipelined): the host sends X one-hot
encoded; 9 accumulating f16 matmuls (lhsT = per-feature embedding table)
produce hT in PSUM directly -- no gathers, no transposes. Three more f16
matmuls with hT as lhsT give node-major q/k/v (a d-major head permutation
and a x64 fp8 scale on Wk are folded into the weights host-side); each kv
row is packed as 384 bytes: 128B k in fp8-e4m3 | 256B v in f16 (k
quantization barely moves the scores; v stays f16 since its error flows
straight into the output).

phase 2: AllGather kv_shard -> kv_full (all 50176 rows on every core).

phase 3 (per tile): K_t single-offset indirect DMAs (128 rows each, the
only gather form this bedrock image supports) fetch the compacted
neighbor rows; DVE attention in f16 with halving-tree reductions
(tensor_tensor keeps the 2x perf mode that tensor_reduce lacks).
Masking: e = exp(s/256) computed first, then e2 = (mask max 1e-4)*e, so
masked lanes get weight ~1e-4/z and a fully-masked row degrades to the
uniform average over its 16 original slots like the jax softmax.
"""
import numpy as np

import concourse.bass as bass
import concourse.mybir as mybir
import concourse.tile as tile
from concourse.vector_clock import ScopedClock

F32 = mybir.dt.float32
F16 = mybir.dt.float16
F8 = mybir.dt.float8e4
U8 = mybir.dt.uint8
I32 = mybir.dt.int32
I16 = mybir.dt.int16
AX = mybir.AxisListType
ALU = mybir.AluOpType
AF = mybir.ActivationFunctionType

N_FEATS, VOCAB, HID, NH, HD, K = 9, 119, 128, 8, 16, 16
VFLAT = N_FEATS * VOCAB          # 1071 embedding rows
P = 128
NCORES = 8
NRC = 6250          # real nodes per core
NPC = 6272          # padded nodes per core (49 x 128)
T = NPC // P        # 49 tiles
NTOT = NPC * NCORES
ROWB = HID + 2 * HID             # kv row bytes: 128B k-fp8 | 256B v-f16
KVS = 64.0                       # fp8 k scale (folded into Wk host-side)

# ---------------------------------------------------------------- walrus fixes
# This walrus build rejects >1 sync-wait command per instruction. Two fixes:
# (1) TileContext tail drain: emit waits as single-wait nops.
# (2) General: split multi-wait instructions in the serialized BIR JSON by
#     inserting single-wait NoOps immediately before them (order preserved).


def _patched_drain_and_barrier(self, tick_clock, wait_clock):
    nc = self.nc
    probe = nc.sync.nop(nofuse=True)
    wait_clock.add_sem_waits(probe.ins, ScopedClock({None: tick_clock.global_clock}))
    waits = list(probe.ins.sync_info.on_wait or []) if probe.ins.sync_info else []
    if probe.ins.sync_info:
        probe.ins.sync_info.on_wait = waits[:1]
    for w in waits[1:]:
        n2 = nc.sync.nop(nofuse=True)
        if n2.ins.sync_info is None:
            n2.ins.sync_info = mybir.SyncInfo(on_update=[], on_wait=[w])
        else:
            n2.ins.sync_info.on_wait = [w]
    nc.sync.drain()
    nc.all_engine_barrier()
    assert self.sems is not None
    popped = nc._tile_sem_poison_stack.pop()
    assert popped is self._sem_poison
    nc.clear_and_free_semaphores(list(self.sems.allocated().values()))
    nc.all_engine_barrier()


tile.TileContext._drain_and_barrier = _patched_drain_and_barrier


def _split_waits_json(bir_bytes):
    import orjson
    m = orjson.loads(bir_bytes)
    n = 0
    for fn in m["functions"]:
        for blk in fn["blocks"]:
            new = []
            for ins in blk["instructions"]:
                si = ins.get("sync_info")
                waits = (si or {}).get("on_wait") or []
                if len(waits) > 1:
                    for w in waits[:-1]:
                        n += 1
                        new.append({
                            "debug": ins.get("debug", 0),
                            "engine": ins["engine"],
                            "ins": [], "name": f"I-wfix-{n}",
                            "opcode": "NoOp", "outs": [],
                            "sync_info": {"on_update": [], "on_wait": [w]},
                        })
                    si["on_wait"] = waits[-1:]
                new.append(ins)
            blk["instructions"] = new
    return orjson.dumps(m), n


import concourse.bass2jax as _b2j

_orig_cbk = _b2j.compile_bir_kernel


def _patched_cbk(ant_bir_str, *a, **kw):
    fixed, n = _split_waits_json(ant_bir_str)
    return _orig_cbk(fixed, *a, **kw)


_b2j.compile_bir_kernel = _patched_cbk

# ---------------------------------------------------------------- device code

# K profile (neighbors kept per tile) is data-dependent; _prep stores it here
# so build() picked up by test.py's bare build(nc) call uses the same one.
_KPROF = [K] * T


def build(nc, kprof=None, phases=3):
    if kprof is None:
        kprof = _KPROF
    SK = sum(kprof)

    oneh = nc.dram_tensor("oneh", [VOCAB, T * N_FEATS * HID], F16,
                          kind="ExternalInput")
    noffs = nc.dram_tensor("noffs", [P, SK], I32, kind="ExternalInput")
    nmask = nc.dram_tensor("nmask", [P, SK], F16, kind="ExternalInput")
    emb = nc.dram_tensor("emb", [VFLAT, HID], F16, kind="ExternalInput")
    wq = nc.dram_tensor("wq", [HID, HID], F16, kind="ExternalInput")
    wk = nc.dram_tensor("wk", [HID, HID], F16, kind="ExternalInput")
    wv = nc.dram_tensor("wv", [HID, HID], F16, kind="ExternalInput")
    out = nc.dram_tensor("out", [NPC, HID], F32, kind="ExternalOutput")

    lp = nc.allow_low_precision(reason="fp16 attention pipeline")
    lp.__enter__()
    with tile.TileContext(nc) as tc:
        with (
            tc.tile_pool(name="const", bufs=1) as cp,
            tc.tile_pool(name="resident", bufs=1) as rp,
            tc.tile_pool(name="work", bufs=3) as wp,
            tc.tile_pool(name="gath", bufs=3) as gp,
            tc.tile_pool(name="kvw", bufs=2) as kp,
            tc.tile_pool(name="psum", bufs=3, space="PSUM") as pp,
            tc.tile_pool(name="dram", bufs=1, space="DRAM") as dp,
        ):
            zeroc = cp.tile([P, 1], F32, name="zeroc")
            nc.gpsimd.memset(zeroc[:], 0.0)
            w_q = cp.tile([HID, HID], F16, name="w_q")
            w_k = cp.tile([HID, HID], F16, name="w_k")
            w_v = cp.tile([HID, HID], F16, name="w_v")
            for t_, d_ in ((w_q, wq), (w_k, wk), (w_v, wv)):
                nc.sync.dma_start(out=t_[:], in_=d_[:])
            embs = cp.tile([VOCAB, N_FEATS * HID], F16, name="embs")
            nc.sync.dma_start(
                out=embs[:].rearrange("v (f c) -> v f c", f=N_FEATS),
                in_=emb[:].rearrange("(f v) c -> v f c", f=N_FEATS))
            nof = rp.tile([P, SK], I32, name="nof")
            nc.sync.dma_start(out=nof[:], in_=noffs[:])
            nmk = rp.tile([P, SK], F16, name="nmk")
            nc.sync.dma_start(out=nmk[:], in_=nmask[:])

            q_all = rp.tile([P, T * HID], F16, name="q_all")

            kv_shard = dp.tile([NPC, ROWB], U8, name="kv_shard")
            kv_full = dp.tile([NTOT, ROWB], U8, name="kv_full",
                              addr_space="Shared")

            # phase 1: one-hot matmul embeddings -> hT -> q,k,v.
            # software-pipelined one tile so the PE never stalls on the
            # Act-engine PSUM evacuation of hT.
            KVB = 4                       # tiles per kv_shard writeback
            hTs = {}
            for t in range(T + 1):
                if t < T and t % 2 == 0:
                    tb = min(2, T - t)       # tiles in this one-hot batch
                    oh = gp.tile([VOCAB, tb * N_FEATS * HID], F16, name="oh")
                    eng = nc.gpsimd if (t // 2) % 2 == 0 else nc.sync
                    eng.dma_start(
                        out=oh[:],
                        in_=oneh[:, t * N_FEATS * HID:
                                 (t + tb) * N_FEATS * HID])
                    hT_p = pp.tile([P, tb * P], F32, name="hT_p",
                                   space="PSUM")
                    for f in range(N_FEATS):
                        nc.tensor.matmul(
                            out=hT_p[:].rearrange("p (b c) -> p b c", b=tb),
                            lhsT=embs[:, f * HID:(f + 1) * HID],
                            rhs=oh[:].rearrange("v (b f c) -> v b f c",
                                                b=tb, f=N_FEATS)
                                [:, :, f, :],
                            start=(f == 0), stop=(f == N_FEATS - 1))
                    hT = wp.tile([P, tb * P], F16, name="hT")
                    nc.scalar.copy(out=hT[:], in_=hT_p[:])
                    for b in range(tb):
                        hTs[t + b] = hT[:, b * P:(b + 1) * P]
                if t < 1:
                    continue
                u = t - 1
                if u % KVB == 0:
                    kvb = min(KVB, T - u)
                    kvt = kp.tile([P, kvb * ROWB], U8, name="kvt")
                hTu = hTs.pop(u)
                for i, wmat in enumerate((w_q, w_k, w_v)):
                    mm = pp.tile([P, P], F32, name="mm", space="PSUM")
                    nc.tensor.matmul(out=mm[:], lhsT=hTu, rhs=wmat[:],
                                     start=True, stop=True)
                    if i == 0:
                        nc.scalar.copy(out=q_all[:, u * HID:(u + 1) * HID],
                                       in_=mm[:])
                    elif i == 1:
                        off = (u % KVB) * ROWB
                        dst = kvt[:, off:off + HID].bitcast(F8)
                        nc.vector.tensor_copy(out=dst, in_=mm[:])
                    else:
                        off = (u % KVB) * ROWB + HID
                        dst = kvt[:, off:off + 2 * HID].bitcast(F16)
                        nc.vector.tensor_copy(out=dst, in_=mm[:])
                if u % KVB == kvb - 1:
                    r0 = (u - kvb + 1) * P
                    nc.sync.dma_start(
                        out=kv_shard[r0:r0 + kvb * P, :]
                            .rearrange("(b p) c -> p b c", b=kvb),
                        in_=kvt[:].rearrange("p (b c) -> p b c", b=kvb))

            # phase 2: allgather kv across the 8 cores
            if phases >= 2:
                nc.gpsimd.collective_compute(
                    "AllGather", ALU.bypass,
                    replica_groups=[list(range(NCORES))],
                    ins=[kv_shard[:]], outs=[kv_full[:]])

            # phase 3: neighbor gather + attention
            ofs = 0
            for t in range(T if phases >= 3 else 0):
                Kt = kprof[t]
                r0 = t * P
                knvn = gp.tile([P, Kt * ROWB], U8, name="knvn")
                for j in range(Kt):
                    nc.gpsimd.indirect_dma_start(
                        out=knvn[:, j * ROWB:(j + 1) * ROWB],
                        out_offset=None, in_=kv_full[:],
                        in_offset=bass.IndirectOffsetOnAxis(
                            ap=nof[:, ofs + j:ofs + j + 1], axis=0))
                kv = knvn[:].rearrange("p (k c) -> p k c", k=Kt)
                kn = kv[:, :, 0:HID].bitcast(F8)
                vn = kv[:, :, HID:ROWB].bitcast(F16)

                qb = q_all[:, t * HID:(t + 1) * HID] \
                    .rearrange("p (a c) -> p a c", a=1).to_broadcast([P, Kt, HID])
                prod = wp.tile([P, Kt * HID], F16, name="prod")
                nc.vector.tensor_tensor(
                    out=prod[:].rearrange("p (k c) -> p k c", k=Kt),
                    in0=kn, in1=qb, op=ALU.mult)

                # score d-reduce: halving tree of tensor_tensor adds (2x mode)
                cur, d = prod, HD
                while d > 1:
                    pv = cur[:].rearrange("p (k d h) -> p k d h", d=d, h=NH)
                    nxt = wp.tile([P, Kt * (d // 2) * NH], F16, name=f"dr{d}")
                    nc.vector.tensor_tensor(
                        out=nxt[:].rearrange("p (k d h) -> p k d h",
                                             d=d // 2, h=NH),
                        in0=pv[:, :, 0:d // 2, :], in1=pv[:, :, d // 2:d, :],
                        op=ALU.add)
                    cur, d = nxt, d // 2
                s = cur

                # e = exp(scores) directly (s = 4*KVS*scores); masked lanes
                # get 1e-4 so a fully-masked row degrades to the uniform
                # average like jax softmax
                e = wp.tile([P, Kt * NH], F16, name="e")
                nc.scalar.activation(out=e[:], in_=s[:], func=AF.Exp,
                                     bias=zeroc[:], scale=1.0 / (4.0 * KVS))
                mb = nmk[:, ofs:ofs + Kt] \
                    .rearrange("p (k a) -> p k a", a=1).to_broadcast([P, Kt, NH])
                e2 = wp.tile([P, Kt * NH], F16, name="e2")
                nc.vector.scalar_tensor_tensor(
                    out=e2[:].rearrange("p (k h) -> p k h", k=Kt),
                    in0=mb, scalar=1e-4,
                    in1=e[:].rearrange("p (k h) -> p k h", k=Kt),
                    op0=ALU.max, op1=ALU.mult)

                z = wp.tile([P, NH], F32, name="z")
                nc.vector.tensor_reduce(
                    out=z[:], in_=e2[:].rearrange("p (k h) -> p h k", h=NH),
                    axis=AX.X, op=ALU.add)
                zr = wp.tile([P, NH], F32, name="zr")
                nc.vector.reciprocal(out=zr[:], in_=z[:])

                at = wp.tile([P, Kt * NH], F16, name="at")
                nc.vector.tensor_tensor(
                    out=at[:].rearrange("p (k h) -> p k h", k=Kt),
                    in0=e2[:].rearrange("p (k h) -> p k h", k=Kt),
                    in1=zr[:].rearrange("p (a h) -> p a h", a=1)
                        .to_broadcast([P, Kt, NH]),
                    op=ALU.mult)

                prod2 = wp.tile([P, Kt * HID], F16, name="prod2")
                nc.vector.tensor_tensor(
                    out=prod2[:].rearrange("p (k d h) -> p k d h", d=HD, h=NH),
                    in0=vn.rearrange("p k (d h) -> p k d h", d=HD),
                    in1=at[:].rearrange("p (k a h) -> p k a h", a=1, h=NH)
                        .to_broadcast([P, Kt, HD, NH]),
                    op=ALU.mult)

                K2 = Kt // 2
                G = K2 + Kt % 2
                p2 = prod2[:].rearrange("p (k c) -> p k c", k=Kt)
                oA = wp.tile([P, G * HID], F16, name="oA")
                nc.vector.tensor_tensor(
                    out=oA[:, 0:K2 * HID].rearrange("p (k c) -> p k c", k=K2),
                    in0=p2[:, 0:K2, :], in1=p2[:, K2:2 * K2, :], op=ALU.add)
                if Kt % 2:
                    nc.vector.tensor_copy(
                        out=oA[:, K2 * HID:G * HID],
                        in_=prod2[:, (Kt - 1) * HID:Kt * HID])

                o = wp.tile([P, HID], F32, name="o")
                nc.vector.tensor_reduce(
                    out=o[:].rearrange("p (h d) -> p d h", h=NH),
                    in_=oA[:].rearrange("p (k d h) -> p d h k", d=HD, h=NH),
                    axis=AX.X, op=ALU.add)
                nc.sync.dma_start(out=out[r0:r0 + P, :], in_=o[:])
                ofs += Kt
    lp.__exit__(None, None, None)
    return nc


# ---------------------------------------------------------------- host side


def _prep(X, nbr_idx, nbr_mask, atom_emb, Wq, bq, Wk, bk, Wv, bv):
    global _KPROF
    X = np.asarray(X).astype(np.int64)
    g = np.asarray(nbr_idx).astype(np.int64)
    mask = np.asarray(nbr_mask).astype(bool)
    N = X.shape[0]

    # d-major head permutation of output columns: col (h*HD+d) -> (d*NH+h)
    perm_cols = (np.arange(HID) % HD) * NH + (np.arange(HID) // HD)
    inv_cols = np.empty(HID, np.int64)
    inv_cols[perm_cols] = np.arange(HID)
    assert not (np.any(np.asarray(bq)) or np.any(np.asarray(bk))
                or np.any(np.asarray(bv))), "nonzero qkv bias unsupported"
    wq_p = np.ascontiguousarray(
        np.asarray(Wq, np.float32)[:, inv_cols]).astype(np.float16)
    wk_p = np.ascontiguousarray(
        (np.asarray(Wk, np.float32) * KVS)[:, inv_cols]).astype(np.float16)
    wv_p = np.ascontiguousarray(
        np.asarray(Wv, np.float32)[:, inv_cols]).astype(np.float16)

    emb_f = np.asarray(atom_emb, np.float32).reshape(VFLAT, HID) \
        .astype(np.float16)

    # global count-rank round-robin: node at rank i -> core i%8, pos i//8.
    # keeps every core's per-tile count profile identical. fully-masked
    # nodes count as 16 so they land in K=16 tiles with all 16 slots.
    counts = mask.sum(1)
    ckey = np.where(counts == 0, K, counts)
    order = np.argsort(ckey, kind="stable")
    core_of = np.empty(N, np.int64)
    pos_of = np.empty(N, np.int64)
    core_of[order] = np.arange(N) % NCORES
    pos_of[order] = np.arange(N) // NCORES
    remap = core_of * NPC + pos_of          # kv_full row per global node

    # per-tile K: max effective count in the tile's global rank band
    kprof = []
    for t in range(T):
        lo, hi = t * P * NCORES, min((t + 1) * P * NCORES, N)
        kt = int(ckey[order[lo:hi]].max()) if hi > lo else 2
        kprof.append(min(K, max(2, kt)))
    _KPROF = kprof
    SK = sum(kprof)

    srcs = []
    maps = []
    for r in range(NCORES):
        # global node id at each core position (or -1 for pads)
        src = np.full(NPC, -1, np.int64)
        mine = order[np.nonzero(core_of[order] == r)[0]]
        src[:mine.size] = mine          # pos_of[mine] == arange(mine.size)
        srcs.append(src)
        real = src >= 0
        Xp = np.zeros((NPC, N_FEATS), np.int64)
        gp_ = np.zeros((NPC, K), np.int64)
        mp = np.zeros((NPC, K), bool)
        Xp[real] = X[src[real]]
        gp_[real] = g[src[real]]
        mp[real] = mask[src[real]]

        # one-hot encoding of X, f16 [119, T*9*128]
        oneh = np.zeros((VOCAB, T * N_FEATS * P), np.float16)
        cols = (np.arange(T)[:, None, None] * N_FEATS * P
                + np.arange(N_FEATS)[None, :, None] * P
                + np.arange(P)[None, None, :])
        rows_oh = Xp.reshape(T, P, N_FEATS).transpose(0, 2, 1)
        oneh[rows_oh.ravel(), cols.ravel()] = 1.0

        # compacted neighbor offsets + masks
        noffs = np.zeros((P, SK), np.int32)
        nmk = np.zeros((P, SK), np.float16)
        rows = remap[gp_.reshape(-1)].reshape(NPC, K).astype(np.int32)
        ofs = 0
        for t in range(T):
            Kt = kprof[t]
            for i in range(P):
                n = t * P + i
                if src[n] < 0:
                    continue
                act = np.nonzero(mp[n])[0]
                if act.size == 0:
                    # fully masked: keep original 16 slots, all mask 0
                    assert Kt == K
                    noffs[i, ofs:ofs + Kt] = rows[n, :Kt]
                else:
                    na = act.size
                    noffs[i, ofs:ofs + na] = rows[n, act]
                    nmk[i, ofs:ofs + na] = 1.0
            ofs += Kt

        maps.append({
            "oneh": oneh, "noffs": noffs, "nmask": nmk, "emb": emb_f,
            "wq": wq_p, "wk": wk_p, "wv": wv_p,
        })
    return maps, srcs


_CACHE = {}


def run_on_device(maps, kprof, trace=False):
    from concourse.bass_utils import run_bass_kernel_spmd
    key = tuple(kprof)
    if key not in _CACHE:
        _CACHE.clear()
        nc = bass.Bass()
        build(nc, list(kprof))
        _CACHE[key] = nc
    return run_bass_kernel_spmd(_CACHE[key], maps, list(range(NCORES)),
                                trace=trace)


def kernel(X, nbr_idx, nbr_mask, atom_emb, Wq, bq, Wk, bk, Wv, bv):
    maps, srcs = _prep(X, nbr_idx, nbr_mask, atom_emb, Wq, bq, Wk, bk, Wv, bv)
    res = run_on_device(maps, _KPROF)
    N = X.shape[0]
    out = np.empty((N, HID), np.float32)
    for r in range(NCORES):
        src = srcs[r]
        real = src >= 0
        out[src[real]] = res.results[r]["out"][real.nonzero()[0]]
    return out
